# revision 38
# baseline (speedup 1.0000x reference)
"""Trainium2 Bass kernel for nn_BiLSTM_M_61615600828569 (segment_reduce).

Full computation per batch:
  span_emb = masked-max-pool of token windows   (B,256,768)
  vertex_emb = masked-mean over coref spans     (B,128,768)
  head/tail  = vertex gather by relation        (B,512,768)
  feat = [head, eh, tail, et, head*tail]        (B,512,2344)
  out  = relu(feat @ W1) @ W2 + b2              (B,512,97)

Sharding: data-parallel over batch; 16 batches / 8 cores = 2 per core.

All index work happens on host, including the span-window gather itself:
for each span [s, e] the host stages 8 token rows
  [s, s+1^e, s+2^e, s+3^e, e-3|s, e-2|s, e-1|s, e]
(clamped duplicates instead of -inf masking -- every staged row is a
valid span member and their union covers [s, e] exactly for any width
0..7).  The device then reduces each span with a pure tensor_tensor max
pyramid -- no dma_gather, no gpsimd descriptor generation, no
scalar_tensor_tensor masking.

Device pipeline per batch: quad DMA chunks land -> DVE max pyramid per
128-span half -> PE: vertex pooling, V_emb^T, head/tail gather,
(V_emb @ W1) blocks, hidden accumulation (with the two distance-embed
tables stacked into a single K=40 matmul), output matmul; each batch's
output is DMA'd out as soon as it is ready.  All float math on device in
bf16 with fp32 PSUM accumulation, transposed layout (features on
partitions) so the final predict.T has the 97 classes on partitions for
a per-partition bias add.
"""
import numpy as np
import ml_dtypes
from contextlib import ExitStack

import concourse.bass as bass
import concourse.bacc as bacc
import concourse.tile as tile
from concourse import mybir
from concourse import bass_utils

BF16 = ml_dtypes.bfloat16

B, S, D = 16, 1024, 768
NS, MAXW = 256, 8
V, C = 128, 6
R = 512
REL, HID, DIS = 97, 384, 20

NCORES = 8
NB = B // NCORES          # batches per core = 2
NM = 6                    # 128-row d-chunks in D
NM3 = HID // 128          # hidden 128-row chunks = 3
NKA = 12                  # w1main chunks for a+c blocks
NKP = 6                   # w1main chunks for prod block
NKMAIN = NKA + NKP        # 18

# early const blob (bf16, lands first): [poolt | disbd | distsel | w1bd]
# rows 0..39 carry the 40-partition dis tables/weights in their own columns.
CA_POOL = 0                       # [128, NB, 2, V]        NB*2*V   = 512
CA_DISBD = CA_POOL + NB * 2 * V   # [40, 40] block-diag dis_embed^T
CA_DSEL = CA_DISBD + 40           # [40, NB, R] stacked ehsel/etsel
CA_W1BD = CA_DSEL + NB * R        # [40, HID] stacked W1b/W1d rows
CA_COLS = CA_W1BD + HID

# late const blob (bf16): [hsel | tsel | w2]
CB_HSEL = 0                       # [128, NB, R]           NB*R     = 1024
CB_TSEL = CB_HSEL + NB * R        # [128, NB, R]           NB*R     = 1024
CB_W2 = CB_TSEL + NB * R          # [128, 3, REL]          3*REL    = 291
CB_COLS = CB_W2 + NM3 * REL

F8 = ml_dtypes.float8_e4m3


def _patch_drain_and_barrier():
    """Walrus rejects >1 explicit sync wait on a Drain (TPB_CTRL), but Tile's
    tail drain waits on every used proc sem at once. Emit one single-wait
    drain per proc instead; the final drain then needs no waits."""
    import concourse.tile as tile_mod
    from concourse.vector_clock import VectorClock, ScopedClock

    if getattr(tile_mod.TileContext, "_ant_drain_patched", False):
        return

    def _patched(self, tick_clock, wait_clock):
        full = tick_clock.global_clock
        n = len(full)
        engines = [self.nc.sync, self.nc.vector, self.nc.scalar,
                   self.nc.tensor, self.nc.gpsimd]
        for i, p in enumerate([q for q in range(n) if full[q] > 0]):
            vec = [full[q] if q == p else 0 for q in range(n)]
            d = engines[i % len(engines)].drain()
            wait_clock.add_sem_waits(d.ins, ScopedClock({None: VectorClock(vec)}))
        self.nc.sync.drain()
        self.nc.all_engine_barrier()
        popped = self.nc._tile_sem_poison_stack.pop()
        assert popped is self._sem_poison
        # outermost (only) tile context at program end: skip the on-device
        # semaphore clear + second barrier; do host bookkeeping only.
        sem_nums = [s.num if hasattr(s, "num") else s
                    for s in self.sems.allocated().values()]
        self.nc._state.prepend_free_semaphores(sem_nums)

    tile_mod.TileContext._drain_and_barrier = _patched
    tile_mod.TileContext._ant_drain_patched = True


_patch_drain_and_barrier()

_NC_CACHE = None


def _build():
    """One-core program; SPMD-replicated across the 8 cores."""
    bf = mybir.dt.bfloat16
    f32 = mybir.dt.float32
    AF = mybir.ActivationFunctionType
    MAX = mybir.AluOpType.max

    nc = bacc.Bacc("TRN2", target_bir_lowering=False, debug=False, num_devices=1)

    # host-staged span windows: per (batch, span-half q, row-half) one
    # contiguous [128, 4*D] chunk; span i = q*128 + p.
    f8 = mybir.dt.float8e4
    DR = mybir.MatmulPerfMode.DoubleRow
    sq = [[[nc.dram_tensor(f"sq_{h}_{q}_{f}", (128, 4 * D), bf,
                           kind="ExternalInput")
            for f in range(2)] for q in range(2)] for h in range(NB)]
    cba = nc.dram_tensor("cba", (128, CA_COLS), bf, kind="ExternalInput")
    cbb = nc.dram_tensor("cbb", (128, CB_COLS), bf, kind="ExternalInput")
    w1ac = nc.dram_tensor("w1ac", (128, NKA, HID), bf, kind="ExternalInput")
    w1p = nc.dram_tensor("w1p", (128, NKP, HID), bf, kind="ExternalInput")
    cb32 = nc.dram_tensor("cb32", (128, NB + 1), f32, kind="ExternalInput")
    outd = [nc.dram_tensor(f"outd{b}", (128, R), f32, kind="ExternalOutput")
            for b in range(NB)]

    with tile.TileContext(nc) as tc, ExitStack() as ctx:
        consts = ctx.enter_context(tc.tile_pool(name="consts", bufs=1))
        work = ctx.enter_context(tc.tile_pool(name="work", bufs=1))
        perb = ctx.enter_context(tc.tile_pool(name="perb", bufs=2))
        psums = ctx.enter_context(tc.tile_pool(name="psums", bufs=1, space="PSUM"))

        def psum_tile(name, tag, bufs, shape=None):
            return psums.tile(shape or [128, R], mybir.dt.float32, space="PSUM",
                              tag=tag, bufs=bufs, name=name)

        # ---- input DMAs, in priority order, all issued on sync ----
        sq_t = [[[None] * 2 for _ in range(2)] for _ in range(NB)]
        for f in range(2):
            t = work.tile([128, 4 * D], bf, name=f"sq_0_0_{f}",
                          tag=f"sq_0_0_{f}")
            nc.sync.dma_start(out=t[:], in_=sq[0][0][f].ap())
            sq_t[0][0][f] = t
        cba_t = consts.tile([128, CA_COLS], bf)
        nc.sync.dma_start(out=cba_t[:], in_=cba.ap())
        cb32_t = consts.tile([128, NB + 1], f32)
        nc.sync.dma_start(out=cb32_t[:], in_=cb32.ap())
        for f in range(2):
            t = work.tile([128, 4 * D], bf, name=f"sq_0_1_{f}",
                          tag=f"sq_0_1_{f}")
            nc.sync.dma_start(out=t[:], in_=sq[0][1][f].ap())
            sq_t[0][1][f] = t
        cb_t = consts.tile([128, CB_COLS], bf)
        nc.sync.dma_start(out=cb_t[:], in_=cbb.ap())
        w1_t = consts.tile([128, NKA, HID], bf)
        nc.sync.dma_start(out=w1_t[:], in_=w1ac.ap())
        w1p_t = consts.tile([128, NKP, HID], bf)
        nc.sync.dma_start(out=w1p_t[:], in_=w1p.ap())
        for q in range(2):
            for f in range(2):
                t = work.tile([128, 4 * D], bf, name=f"sq_1_{q}_{f}",
                              tag=f"sq_1_{q}_{f}")
                nc.sync.dma_start(out=t[:], in_=sq[1][q][f].ap())
                sq_t[1][q][f] = t

        # const views
        poolt = cba_t[:, CA_POOL : CA_POOL + NB * 2 * V].rearrange(
            "p (b q v) -> p b q v", b=NB, q=2)
        disbd = cba_t[:40, CA_DISBD : CA_DISBD + 40]
        dsel = cba_t[:40, CA_DSEL : CA_DSEL + NB * R].rearrange(
            "p (b r) -> p b r", b=NB)
        w1bd_t = cba_t[:40, CA_W1BD : CA_W1BD + HID]
        hsel = cb_t[:, CB_HSEL : CB_HSEL + NB * R].rearrange(
            "p (b r) -> p b r", b=NB)
        tsel = cb_t[:, CB_TSEL : CB_TSEL + NB * R].rearrange(
            "p (b r) -> p b r", b=NB)
        w2sb = cb_t[:, CB_W2 : CB_W2 + NM3 * REL].rearrange(
            "p (k r) -> p k r", k=NM3)
        inv_t = cb32_t[:, 0:NB]
        b2_t = cb32_t[:, NB : NB + 1]

        # ---- per-batch: span max pyramid then compute ----
        sem_b = []  # sem_b[h][p, q, :] = span_emb[q*128 + p]
        for b in range(NB):
            sh = work.tile([128, 2, D], bf, name=f"sem_{b}", tag=f"sem_{b}")
            for q in range(2):
                t1 = work.tile([128, 4 * D], bf, name=f"t1_{b}_{q}", tag="t1",
                               bufs=2)
                nc.vector.tensor_tensor(out=t1[:], in0=sq_t[b][q][0][:],
                                        in1=sq_t[b][q][1][:], op=MAX)
                nc.vector.tensor_tensor(out=t1[:, 0 : 2 * D],
                                        in0=t1[:, 0 : 2 * D],
                                        in1=t1[:, 2 * D : 4 * D], op=MAX)
                nc.vector.tensor_tensor(out=sh[:, q, :], in0=t1[:, 0:D],
                                        in1=t1[:, D : 2 * D], op=MAX)
            sem_b.append(sh)

            # vertex pooling: V_emb = poolt^T @ span_emb (then * inv)
            ps_v = psums.tile([128, D], mybir.dt.float32, space="PSUM",
                              tag="ps_v", bufs=1, name="ps_v")
            for q in range(2):
                for n0, nsz in ((0, 512), (512, 256)):
                    nc.tensor.matmul(
                        ps_v[:, n0 : n0 + nsz],
                        lhsT=poolt[:, b, q, :],
                        rhs=sem_b[b][:, q, n0 : n0 + nsz],
                        start=(q == 0), stop=(q == 1),
                    )
            v_sb = perb.tile([V, D], bf, tag="v_sb")
            nc.scalar.activation(v_sb[:], ps_v[:], AF.Copy,
                                 scale=inv_t[:, b : b + 1])

            # V_emb^T chunks (unscaled; vw applies inv)
            vt_sb = perb.tile([128, NM, V], bf, tag="vt_sb")
            for m in range(NM):
                ps_vt = psum_tile("ps_vt", "sel", 3)
                for q in range(2):
                    nc.tensor.matmul(ps_vt[:, :V],
                                     lhsT=sem_b[b][:, q, m * 128 : (m + 1) * 128],
                                     rhs=poolt[:, b, q, :],
                                     start=(q == 0), stop=(q == 1))
                nc.any.tensor_copy(vt_sb[:, m, :], ps_vt[:, :V])

            # head/tail gather + product; head factor read straight from
            # PSUM by the multiply (only tail needs an SBUF copy)
            tail_t = perb.tile([128, NM, R], bf, tag="tail_t")
            prod_t = perb.tile([128, NM, R], bf, tag="prod_t")
            for m in range(NM):
                ps_t2 = psum_tile("ps_t2", "sel", 3)
                nc.tensor.matmul(ps_t2[:], lhsT=v_sb[:, m * 128 : (m + 1) * 128],
                                 rhs=tsel[:, b, :], start=True, stop=True)
                nc.any.tensor_copy(tail_t[:, m, :], ps_t2[:])
                ps_h = psum_tile("ps_h", "sel", 3)
                nc.tensor.matmul(ps_h[:], lhsT=v_sb[:, m * 128 : (m + 1) * 128],
                                 rhs=hsel[:, b, :], start=True, stop=True)
                nc.vector.tensor_tensor(out=prod_t[:, m, :],
                                        in0=ps_h[:], in1=tail_t[:, m, :],
                                        op=mybir.AluOpType.mult)

            if b == 0:
                # EwS = blockdiag(disT, disT) @ [W1b; W1d]  (40, HID)
                ps_e = psum_tile("ps_e", "out", 1, shape=[40, HID])
                nc.tensor.matmul(ps_e[:], lhsT=disbd, rhs=w1bd_t,
                                 start=True, stop=True)
                ews = consts.tile([40, HID], bf, name="ews")
                nc.scalar.activation(ews[:], ps_e[:], AF.Copy)

            # Vw_a / Vw_c = (V_emb @ W1a|W1c) * inv
            vw_a = perb.tile([V, HID], bf, tag="vw_a")
            vw_c = perb.tile([V, HID], bf, tag="vw_c")
            ps_vw_a = psum_tile("ps_vw_a", "hid", 2)
            ps_vw_c = psum_tile("ps_vw_c", "hid", 2)
            for m in range(NM):
                nc.tensor.matmul(ps_vw_a[:, :HID], lhsT=vt_sb[:, m, :],
                                 rhs=w1_t[:, m, :],
                                 start=(m == 0), stop=(m == NM - 1))
                nc.tensor.matmul(ps_vw_c[:, :HID], lhsT=vt_sb[:, m, :],
                                 rhs=w1_t[:, NM + m, :],
                                 start=(m == 0), stop=(m == NM - 1))
            nc.scalar.activation(vw_a[:], ps_vw_a[:, :HID], AF.Copy,
                                 scale=inv_t[:, b : b + 1])
            nc.scalar.activation(vw_c[:], ps_vw_c[:, :HID], AF.Copy,
                                 scale=inv_t[:, b : b + 1])

            # hidden = relu( vw_a[h] + vw_c[t] + EwS-gather + W1p^T prod )
            hid_t = perb.tile([128, NM3, R], bf, tag="hid_t")
            for m3 in range(NM3):
                msl = slice(m3 * 128, (m3 + 1) * 128)
                ps_hid = psum_tile("ps_hid", "hid", 2)
                nc.tensor.matmul(ps_hid[:], lhsT=ews[:, msl], rhs=dsel[:, b, :],
                                 start=True, stop=False)
                nc.tensor.matmul(ps_hid[:], lhsT=vw_a[:, msl], rhs=hsel[:, b, :],
                                 start=False, stop=False)
                nc.tensor.matmul(ps_hid[:], lhsT=vw_c[:, msl], rhs=tsel[:, b, :],
                                 start=False, stop=False)
                for m in range(NM):
                    nc.tensor.matmul(ps_hid[:], lhsT=w1p_t[:, m, msl],
                                     rhs=prod_t[:, m, :],
                                     start=False, stop=(m == NM - 1))
                nc.scalar.activation(hid_t[:, m3, :], ps_hid[:], AF.Relu)

            # out = W2^T @ hidden + b2, classes on partitions
            out_sb = perb.tile([128, R], f32, tag="out_sb")
            ps_o = psum_tile("ps_o", "out", 1)
            for kc in range(NM3):
                nc.tensor.matmul(ps_o[:REL, :], lhsT=w2sb[:, kc, :],
                                 rhs=hid_t[:, kc, :],
                                 start=(kc == 0), stop=(kc == NM3 - 1))
            nc.scalar.activation(out_sb[:REL, :], ps_o[:REL, :], AF.Identity,
                                 bias=b2_t[:REL, 0:1])
            nc.sync.dma_start(out=outd[b].ap(), in_=out_sb[:])

    nc.compile()
    return nc


def _prep_core(c, sentence_repr, esi, vidx, vmask, ht, dis_h, dis_t,
               dis_embed, w1ac_p, w1p_p, w1bd_p, w2_p, b2):
    """Build the per-core input map for batches [c*NB, c*NB+NB)."""
    bs = range(c * NB, c * NB + NB)
    inputs = {"w1ac": w1ac_p, "w1p": w1p_p}

    poolt = np.zeros((128, NB, 2, V), dtype=BF16)
    hsel = np.zeros((V, NB, R), dtype=BF16)
    tsel = np.zeros((V, NB, R), dtype=BF16)
    dsel = np.zeros((40, NB, R), dtype=BF16)
    invcnt = np.zeros((V, NB), dtype=np.float32)
    rr = np.arange(R)

    for j, b in enumerate(bs):
        # staged span windows: 8 valid (clamped-duplicate) rows per span
        s = esi[b, :, 0]
        e = esi[b, :, 1]
        rows = np.stack([s,
                         np.minimum(s + 1, e),
                         np.minimum(s + 2, e),
                         np.minimum(s + 3, e),
                         np.maximum(e - 3, s),
                         np.maximum(e - 2, s),
                         np.maximum(e - 1, s),
                         e], axis=1)                     # (NS, 8)
        gath = sentence_repr[b][rows]                    # (NS, 8, D) f32
        gath = gath.reshape(2, 128, 2, 4 * D).astype(BF16)   # q, p, f, :
        for q in range(2):
            for f in range(2):
                inputs[f"sq_{j}_{q}_{f}"] = np.ascontiguousarray(gath[q, :, f])

        pt = np.zeros((NS, V), dtype=np.float32)
        np.add.at(pt, (vidx[b].ravel(), np.repeat(np.arange(V), C)),
                  vmask[b].ravel().astype(np.float32))
        poolt[:, j] = pt.reshape(2, 128, V).transpose(1, 0, 2).astype(BF16)
        invcnt[:, j] = 1.0 / np.maximum(vmask[b].sum(axis=1).astype(np.float32), 1.0)
        hsel[ht[b, :, 0], j, rr] = BF16(1.0)
        tsel[ht[b, :, 1], j, rr] = BF16(1.0)
        dsel[dis_h[b], j, rr] = BF16(1.0)
        dsel[20 + dis_t[b], j, rr] = BF16(1.0)

    cba = np.zeros((128, CA_COLS), dtype=BF16)
    cba[:, CA_POOL : CA_POOL + NB * 2 * V] = poolt.reshape(128, -1)
    cba[:20, CA_DISBD : CA_DISBD + 20] = dis_embed.T.astype(BF16)
    cba[20:40, CA_DISBD + 20 : CA_DISBD + 40] = dis_embed.T.astype(BF16)
    cba[:40, CA_DSEL : CA_DSEL + NB * R] = dsel.reshape(40, -1)
    cba[:40, CA_W1BD : CA_W1BD + HID] = w1bd_p
    inputs["cba"] = cba

    cbb = np.zeros((128, CB_COLS), dtype=BF16)
    cbb[:V, CB_HSEL : CB_HSEL + NB * R] = hsel.reshape(V, -1)
    cbb[:V, CB_TSEL : CB_TSEL + NB * R] = tsel.reshape(V, -1)
    cbb[:, CB_W2 : CB_W2 + NM3 * REL] = w2_p.reshape(128, -1)
    inputs["cbb"] = cbb

    cb32 = np.zeros((128, NB + 1), dtype=np.float32)
    cb32[:V, 0:NB] = invcnt
    cb32[:REL, NB] = b2
    inputs["cb32"] = cb32
    return inputs


def run(trace=False, **inputs):
    global _NC_CACHE
    sentence_repr = np.asarray(inputs["sentence_repr"], dtype=np.float32)
    esi = np.asarray(inputs["entity_span_indices"]).astype(np.int64)
    vidx = np.asarray(inputs["vertex_indices"]).astype(np.int64)
    vmask = np.asarray(inputs["vertex_indices_mask"]).astype(np.int64)
    ht = np.asarray(inputs["head_tail_indices"]).astype(np.int64)
    dis_h = np.asarray(inputs["dis_h_2_t"]).astype(np.int64)
    dis_t = np.asarray(inputs["dis_t_2_h"]).astype(np.int64)
    dis_embed = np.asarray(inputs["dis_embed"], dtype=np.float32)
    w1 = np.asarray(inputs["W1"], dtype=np.float32)
    w2 = np.asarray(inputs["W2"], dtype=np.float32)
    b2 = np.asarray(inputs["b2"], dtype=np.float32)

    # W1 row blocks: a (0:768) -> w1ac chunks 0-5, c (788:1556) -> 6-11,
    # p (1576:2344) -> w1p chunks 0-5; b (768:788) + d (1556:1576) -> w1bd.
    w1ac_rows = np.concatenate([w1[0:768], w1[788:1556]])
    w1ac_p = np.ascontiguousarray(
        w1ac_rows.astype(BF16).reshape(NKA, 128, HID).transpose(1, 0, 2))
    w1p_p = np.ascontiguousarray(
        w1[1576:2344].astype(BF16).reshape(NKP, 128, HID).transpose(1, 0, 2))
    w1bd_p = np.ascontiguousarray(
        np.concatenate([w1[768:788], w1[1556:1576]]).astype(BF16))
    w2_p = np.ascontiguousarray(
        w2.astype(BF16).reshape(NM3, 128, REL).transpose(1, 0, 2))

    in_maps = [
        _prep_core(c, sentence_repr, esi, vidx, vmask, ht, dis_h, dis_t,
                   dis_embed, w1ac_p, w1p_p, w1bd_p, w2_p, b2)
        for c in range(NCORES)
    ]

    if _NC_CACHE is None:
        _NC_CACHE = _build()

    res = bass_utils.run_bass_kernel_spmd(
        _NC_CACHE, in_maps, core_ids=list(range(NCORES)), trace=trace
    )

    out = np.empty((B, R, REL), dtype=np.float32)
    for c in range(NCORES):
        for j in range(NB):
            o = np.asarray(res.results[c][f"outd{j}"], dtype=np.float32)
            out[c * NB + j] = o[:REL].T
    return out, res


def kernel(**inputs):
    out, _ = run(**inputs)
    return out


# revision 41
# speedup vs baseline: 1.0399x; 1.0399x over previous
"""Trainium2 Bass kernel for nn_BiLSTM_M_61615600828569 (segment_reduce).

Full computation per batch:
  span_emb = masked-max-pool of token windows   (B,256,768)
  vertex_emb = masked-mean over coref spans     (B,128,768)
  head/tail  = vertex gather by relation        (B,512,768)
  feat = [head, eh, tail, et, head*tail]        (B,512,2344)
  out  = relu(feat @ W1) @ W2 + b2              (B,512,97)

Sharding: data-parallel over batch; 16 batches / 8 cores = 2 per core.

All index work happens on host, including the span-window gather itself:
for each span [s, e] the host stages 8 token rows
  [s, s+1^e, s+2^e, s+3^e, e-3|s, e-2|s, e-1|s, e]
(clamped duplicates instead of -inf masking -- every staged row is a
valid span member and their union covers [s, e] exactly for any width
0..7).  The device then reduces each span with a pure tensor_tensor max
pyramid -- no dma_gather, no gpsimd descriptor generation, no
scalar_tensor_tensor masking.

Device pipeline per batch: quad DMA chunks land -> DVE max pyramid per
128-span half -> PE: vertex pooling, V_emb^T, head/tail gather,
(V_emb @ W1) blocks, hidden accumulation (with the two distance-embed
tables stacked into a single K=40 matmul), output matmul; each batch's
output is DMA'd out as soon as it is ready.  All float math on device in
bf16 with fp32 PSUM accumulation, transposed layout (features on
partitions) so the final predict.T has the 97 classes on partitions for
a per-partition bias add.
"""
import numpy as np
import ml_dtypes
from contextlib import ExitStack

import concourse.bass as bass
import concourse.bacc as bacc
import concourse.tile as tile
from concourse import mybir
from concourse import bass_utils

BF16 = ml_dtypes.bfloat16

B, S, D = 16, 1024, 768
NS, MAXW = 256, 8
V, C = 128, 6
R = 512
REL, HID, DIS = 97, 384, 20

NCORES = 8
NB = B // NCORES          # batches per core = 2
NM = 6                    # 128-row d-chunks in D
NM3 = HID // 128          # hidden 128-row chunks = 3
NKA = 12                  # w1main chunks for a+c blocks
NKP = 6                   # w1main chunks for prod block
NKMAIN = NKA + NKP        # 18

# early const blob (bf16, lands first): [poolt | disbd | distsel | w1bd]
# rows 0..39 carry the 40-partition dis tables/weights in their own columns.
CA_POOL = 0                       # [128, NB, 2, V]        NB*2*V   = 512
CA_DISBD = CA_POOL + NB * 2 * V   # [40, 40] block-diag dis_embed^T
CA_DSEL = CA_DISBD + 40           # [40, NB, R] stacked ehsel/etsel
CA_W1BD = CA_DSEL + NB * R        # [40, HID] stacked W1b/W1d rows
CA_COLS = CA_W1BD + HID

# late const blob (bf16): [hsel | tsel | w2]
CB_HSEL = 0                       # [128, NB, R]           NB*R     = 1024
CB_TSEL = CB_HSEL + NB * R        # [128, NB, R]           NB*R     = 1024
CB_W2 = CB_TSEL + NB * R          # [128, 3, REL]          3*REL    = 291
CB_COLS = CB_W2 + NM3 * REL

F8 = ml_dtypes.float8_e4m3


def _patch_drain_and_barrier():
    """Walrus rejects >1 explicit sync wait on a Drain (TPB_CTRL), but Tile's
    tail drain waits on every used proc sem at once. Emit one single-wait
    drain per proc instead; the final drain then needs no waits."""
    import concourse.tile as tile_mod
    from concourse.vector_clock import VectorClock, ScopedClock

    if getattr(tile_mod.TileContext, "_ant_drain_patched", False):
        return

    def _patched(self, tick_clock, wait_clock):
        full = tick_clock.global_clock
        n = len(full)
        engines = [self.nc.sync, self.nc.vector, self.nc.scalar,
                   self.nc.tensor, self.nc.gpsimd]
        for i, p in enumerate([q for q in range(n) if full[q] > 0]):
            vec = [full[q] if q == p else 0 for q in range(n)]
            d = engines[i % len(engines)].drain()
            wait_clock.add_sem_waits(d.ins, ScopedClock({None: VectorClock(vec)}))
        self.nc.sync.drain()
        self.nc.all_engine_barrier()
        popped = self.nc._tile_sem_poison_stack.pop()
        assert popped is self._sem_poison
        # outermost (only) tile context at program end: skip the on-device
        # semaphore clear + second barrier; do host bookkeeping only.
        sem_nums = [s.num if hasattr(s, "num") else s
                    for s in self.sems.allocated().values()]
        self.nc._state.prepend_free_semaphores(sem_nums)

    tile_mod.TileContext._drain_and_barrier = _patched
    tile_mod.TileContext._ant_drain_patched = True


_patch_drain_and_barrier()

_NC_CACHE = None


def _build():
    """One-core program; SPMD-replicated across the 8 cores."""
    bf = mybir.dt.bfloat16
    f32 = mybir.dt.float32
    AF = mybir.ActivationFunctionType
    MAX = mybir.AluOpType.max

    nc = bacc.Bacc("TRN2", target_bir_lowering=False, debug=False, num_devices=1)

    # host-staged span windows: per (batch, span-half q, row-half) one
    # contiguous [128, 4*D] chunk; span i = q*128 + p.
    f8 = mybir.dt.float8e4
    DR = mybir.MatmulPerfMode.DoubleRow
    sq = [[[nc.dram_tensor(f"sq_{h}_{q}_{f}", (128, 4 * D), bf,
                           kind="ExternalInput")
            for f in range(2)] for q in range(2)] for h in range(NB)]
    cba = nc.dram_tensor("cba", (128, CA_COLS), bf, kind="ExternalInput")
    cbb = nc.dram_tensor("cbb", (128, CB_COLS), bf, kind="ExternalInput")
    w1ac = nc.dram_tensor("w1ac", (128, NKA, HID), bf, kind="ExternalInput")
    w1p = nc.dram_tensor("w1p", (128, NKP, HID), bf, kind="ExternalInput")
    cb32 = nc.dram_tensor("cb32", (128, NB + 1), f32, kind="ExternalInput")
    outd = [nc.dram_tensor(f"outd{b}", (128, R), f32, kind="ExternalOutput")
            for b in range(NB)]

    with tile.TileContext(nc) as tc, ExitStack() as ctx:
        consts = ctx.enter_context(tc.tile_pool(name="consts", bufs=1))
        work = ctx.enter_context(tc.tile_pool(name="work", bufs=1))
        perb = ctx.enter_context(tc.tile_pool(name="perb", bufs=2))
        psums = ctx.enter_context(tc.tile_pool(name="psums", bufs=1, space="PSUM"))

        def psum_tile(name, tag, bufs, shape=None):
            return psums.tile(shape or [128, R], mybir.dt.float32, space="PSUM",
                              tag=tag, bufs=bufs, name=name)

        # ---- input DMAs, in priority order, all issued on sync ----
        sq_t = [[[None] * 2 for _ in range(2)] for _ in range(NB)]
        for f in range(2):
            t = work.tile([128, 4 * D], bf, name=f"sq_0_0_{f}",
                          tag=f"sq_0_0_{f}")
            nc.sync.dma_start(out=t[:], in_=sq[0][0][f].ap())
            sq_t[0][0][f] = t
        cba_t = consts.tile([128, CA_COLS], bf)
        nc.sync.dma_start(out=cba_t[:], in_=cba.ap())
        cb32_t = consts.tile([128, NB + 1], f32)
        nc.sync.dma_start(out=cb32_t[:], in_=cb32.ap())
        for f in range(2):
            t = work.tile([128, 4 * D], bf, name=f"sq_0_1_{f}",
                          tag=f"sq_0_1_{f}")
            nc.sync.dma_start(out=t[:], in_=sq[0][1][f].ap())
            sq_t[0][1][f] = t
        cb_t = consts.tile([128, CB_COLS], bf)
        nc.sync.dma_start(out=cb_t[:], in_=cbb.ap())
        w1_t = consts.tile([128, NKA, HID], bf)
        nc.sync.dma_start(out=w1_t[:], in_=w1ac.ap())
        w1p_t = consts.tile([128, NKP, HID], bf)
        nc.sync.dma_start(out=w1p_t[:], in_=w1p.ap())
        for q in range(2):
            for f in range(2):
                t = work.tile([128, 4 * D], bf, name=f"sq_1_{q}_{f}",
                              tag=f"sq_1_{q}_{f}")
                nc.sync.dma_start(out=t[:], in_=sq[1][q][f].ap())
                sq_t[1][q][f] = t

        # const views
        poolt = cba_t[:, CA_POOL : CA_POOL + NB * 2 * V].rearrange(
            "p (b q v) -> p b q v", b=NB, q=2)
        disbd = cba_t[:40, CA_DISBD : CA_DISBD + 40]
        dsel = cba_t[:40, CA_DSEL : CA_DSEL + NB * R].rearrange(
            "p (b r) -> p b r", b=NB)
        w1bd_t = cba_t[:40, CA_W1BD : CA_W1BD + HID]
        hsel = cb_t[:, CB_HSEL : CB_HSEL + NB * R].rearrange(
            "p (b r) -> p b r", b=NB)
        tsel = cb_t[:, CB_TSEL : CB_TSEL + NB * R].rearrange(
            "p (b r) -> p b r", b=NB)
        w2sb = cb_t[:, CB_W2 : CB_W2 + NM3 * REL].rearrange(
            "p (k r) -> p k r", k=NM3)
        inv_t = cb32_t[:, 0:NB]
        b2_t = cb32_t[:, NB : NB + 1]

        # ---- PE warm-up: keep the tensor engine busy (and its clock
        # ramping) while the first batch's span data is still in flight ----
        for _ in range(44):
            ps_w = psum_tile("ps_w", "out", 1, shape=[40, 64])
            nc.tensor.matmul(ps_w[:], lhsT=disbd, rhs=w1bd_t[:, 0:64],
                             start=True, stop=True)

        # ---- EwS = blockdiag(disT, disT) @ [W1b; W1d]  (40, HID) ----
        ps_e = psum_tile("ps_e", "out", 1, shape=[40, HID])
        nc.tensor.matmul(ps_e[:], lhsT=disbd, rhs=w1bd_t,
                         start=True, stop=True)
        ews = consts.tile([40, HID], bf, name="ews")
        nc.scalar.activation(ews[:], ps_e[:], AF.Copy)

        # ---- per-batch: span max pyramid then compute ----
        sem_b = []  # sem_b[h][p, q, :] = span_emb[q*128 + p]
        for b in range(NB):
            sh = work.tile([128, 2, D], bf, name=f"sem_{b}", tag=f"sem_{b}")
            for q in range(2):
                t1 = work.tile([128, 4 * D], bf, name=f"t1_{b}_{q}", tag="t1",
                               bufs=2)
                nc.vector.tensor_tensor(out=t1[:], in0=sq_t[b][q][0][:],
                                        in1=sq_t[b][q][1][:], op=MAX)
                nc.vector.tensor_tensor(out=t1[:, 0 : 2 * D],
                                        in0=t1[:, 0 : 2 * D],
                                        in1=t1[:, 2 * D : 4 * D], op=MAX)
                nc.vector.tensor_tensor(out=sh[:, q, :], in0=t1[:, 0:D],
                                        in1=t1[:, D : 2 * D], op=MAX)
            sem_b.append(sh)

            # vertex pooling: V_emb = poolt^T @ span_emb (then * inv)
            ps_v = psums.tile([128, D], mybir.dt.float32, space="PSUM",
                              tag="ps_v", bufs=1, name="ps_v")
            for q in range(2):
                for n0, nsz in ((0, 512), (512, 256)):
                    nc.tensor.matmul(
                        ps_v[:, n0 : n0 + nsz],
                        lhsT=poolt[:, b, q, :],
                        rhs=sem_b[b][:, q, n0 : n0 + nsz],
                        start=(q == 0), stop=(q == 1),
                    )
            v_sb = perb.tile([V, D], bf, tag="v_sb")
            nc.scalar.activation(v_sb[:], ps_v[:], AF.Copy,
                                 scale=inv_t[:, b : b + 1])

            # V_emb^T chunks (unscaled; vw applies inv)
            vt_sb = perb.tile([128, NM, V], bf, tag="vt_sb")
            for m in range(NM):
                ps_vt = psum_tile("ps_vt", "sel", 3)
                for q in range(2):
                    nc.tensor.matmul(ps_vt[:, :V],
                                     lhsT=sem_b[b][:, q, m * 128 : (m + 1) * 128],
                                     rhs=poolt[:, b, q, :],
                                     start=(q == 0), stop=(q == 1))
                nc.any.tensor_copy(vt_sb[:, m, :], ps_vt[:, :V])

            # head/tail gather + product
            head_t = perb.tile([128, NM, R], bf, tag="head_t")
            tail_t = perb.tile([128, NM, R], bf, tag="tail_t")
            prod_t = perb.tile([128, NM, R], bf, tag="prod_t")
            for m in range(NM):
                ps_h = psum_tile("ps_h", "sel", 3)
                nc.tensor.matmul(ps_h[:], lhsT=v_sb[:, m * 128 : (m + 1) * 128],
                                 rhs=hsel[:, b, :], start=True, stop=True)
                nc.any.tensor_copy(head_t[:, m, :], ps_h[:])
                ps_t2 = psum_tile("ps_t2", "sel", 3)
                nc.tensor.matmul(ps_t2[:], lhsT=v_sb[:, m * 128 : (m + 1) * 128],
                                 rhs=tsel[:, b, :], start=True, stop=True)
                nc.any.tensor_copy(tail_t[:, m, :], ps_t2[:])
                nc.vector.tensor_tensor(out=prod_t[:, m, :],
                                        in0=head_t[:, m, :],
                                        in1=tail_t[:, m, :],
                                        op=mybir.AluOpType.mult)

            # Vw_a / Vw_c = (V_emb @ W1a|W1c) * inv
            vw_a = perb.tile([V, HID], bf, tag="vw_a")
            vw_c = perb.tile([V, HID], bf, tag="vw_c")
            ps_vw_a = psum_tile("ps_vw_a", "hid", 2)
            ps_vw_c = psum_tile("ps_vw_c", "hid", 2)
            for m in range(NM):
                nc.tensor.matmul(ps_vw_a[:, :HID], lhsT=vt_sb[:, m, :],
                                 rhs=w1_t[:, m, :],
                                 start=(m == 0), stop=(m == NM - 1))
                nc.tensor.matmul(ps_vw_c[:, :HID], lhsT=vt_sb[:, m, :],
                                 rhs=w1_t[:, NM + m, :],
                                 start=(m == 0), stop=(m == NM - 1))
            nc.scalar.activation(vw_a[:], ps_vw_a[:, :HID], AF.Copy,
                                 scale=inv_t[:, b : b + 1])
            nc.scalar.activation(vw_c[:], ps_vw_c[:, :HID], AF.Copy,
                                 scale=inv_t[:, b : b + 1])

            # hidden = relu( vw_a[h] + vw_c[t] + EwS-gather + W1p^T prod )
            hid_t = perb.tile([128, NM3, R], bf, tag="hid_t")
            for m3 in range(NM3):
                msl = slice(m3 * 128, (m3 + 1) * 128)
                ps_hid = psum_tile("ps_hid", "hid", 2)
                nc.tensor.matmul(ps_hid[:], lhsT=ews[:, msl], rhs=dsel[:, b, :],
                                 start=True, stop=False)
                nc.tensor.matmul(ps_hid[:], lhsT=vw_a[:, msl], rhs=hsel[:, b, :],
                                 start=False, stop=False)
                nc.tensor.matmul(ps_hid[:], lhsT=vw_c[:, msl], rhs=tsel[:, b, :],
                                 start=False, stop=False)
                for m in range(NM):
                    nc.tensor.matmul(ps_hid[:], lhsT=w1p_t[:, m, msl],
                                     rhs=prod_t[:, m, :],
                                     start=False, stop=(m == NM - 1))
                nc.scalar.activation(hid_t[:, m3, :], ps_hid[:], AF.Relu)

            # out = W2^T @ hidden + b2, classes on partitions
            out_sb = perb.tile([128, R], f32, tag="out_sb")
            ps_o = psum_tile("ps_o", "out", 1)
            for kc in range(NM3):
                nc.tensor.matmul(ps_o[:REL, :], lhsT=w2sb[:, kc, :],
                                 rhs=hid_t[:, kc, :],
                                 start=(kc == 0), stop=(kc == NM3 - 1))
            nc.scalar.activation(out_sb[:REL, :], ps_o[:REL, :], AF.Identity,
                                 bias=b2_t[:REL, 0:1])
            nc.sync.dma_start(out=outd[b].ap(), in_=out_sb[:])

    nc.compile()
    return nc


def _prep_core(c, sentence_repr, esi, vidx, vmask, ht, dis_h, dis_t,
               dis_embed, w1ac_p, w1p_p, w1bd_p, w2_p, b2):
    """Build the per-core input map for batches [c*NB, c*NB+NB)."""
    bs = range(c * NB, c * NB + NB)
    inputs = {"w1ac": w1ac_p, "w1p": w1p_p}

    poolt = np.zeros((128, NB, 2, V), dtype=BF16)
    hsel = np.zeros((V, NB, R), dtype=BF16)
    tsel = np.zeros((V, NB, R), dtype=BF16)
    dsel = np.zeros((40, NB, R), dtype=BF16)
    invcnt = np.zeros((V, NB), dtype=np.float32)
    rr = np.arange(R)

    for j, b in enumerate(bs):
        # staged span windows: 8 valid (clamped-duplicate) rows per span
        s = esi[b, :, 0]
        e = esi[b, :, 1]
        rows = np.stack([s,
                         np.minimum(s + 1, e),
                         np.minimum(s + 2, e),
                         np.minimum(s + 3, e),
                         np.maximum(e - 3, s),
                         np.maximum(e - 2, s),
                         np.maximum(e - 1, s),
                         e], axis=1)                     # (NS, 8)
        gath = sentence_repr[b][rows]                    # (NS, 8, D) f32
        gath = gath.reshape(2, 128, 2, 4 * D).astype(BF16)   # q, p, f, :
        for q in range(2):
            for f in range(2):
                inputs[f"sq_{j}_{q}_{f}"] = np.ascontiguousarray(gath[q, :, f])

        pt = np.zeros((NS, V), dtype=np.float32)
        np.add.at(pt, (vidx[b].ravel(), np.repeat(np.arange(V), C)),
                  vmask[b].ravel().astype(np.float32))
        poolt[:, j] = pt.reshape(2, 128, V).transpose(1, 0, 2).astype(BF16)
        invcnt[:, j] = 1.0 / np.maximum(vmask[b].sum(axis=1).astype(np.float32), 1.0)
        hsel[ht[b, :, 0], j, rr] = BF16(1.0)
        tsel[ht[b, :, 1], j, rr] = BF16(1.0)
        dsel[dis_h[b], j, rr] = BF16(1.0)
        dsel[20 + dis_t[b], j, rr] = BF16(1.0)

    cba = np.zeros((128, CA_COLS), dtype=BF16)
    cba[:, CA_POOL : CA_POOL + NB * 2 * V] = poolt.reshape(128, -1)
    cba[:20, CA_DISBD : CA_DISBD + 20] = dis_embed.T.astype(BF16)
    cba[20:40, CA_DISBD + 20 : CA_DISBD + 40] = dis_embed.T.astype(BF16)
    cba[:40, CA_DSEL : CA_DSEL + NB * R] = dsel.reshape(40, -1)
    cba[:40, CA_W1BD : CA_W1BD + HID] = w1bd_p
    inputs["cba"] = cba

    cbb = np.zeros((128, CB_COLS), dtype=BF16)
    cbb[:V, CB_HSEL : CB_HSEL + NB * R] = hsel.reshape(V, -1)
    cbb[:V, CB_TSEL : CB_TSEL + NB * R] = tsel.reshape(V, -1)
    cbb[:, CB_W2 : CB_W2 + NM3 * REL] = w2_p.reshape(128, -1)
    inputs["cbb"] = cbb

    cb32 = np.zeros((128, NB + 1), dtype=np.float32)
    cb32[:V, 0:NB] = invcnt
    cb32[:REL, NB] = b2
    inputs["cb32"] = cb32
    return inputs


def run(trace=False, **inputs):
    global _NC_CACHE
    sentence_repr = np.asarray(inputs["sentence_repr"], dtype=np.float32)
    esi = np.asarray(inputs["entity_span_indices"]).astype(np.int64)
    vidx = np.asarray(inputs["vertex_indices"]).astype(np.int64)
    vmask = np.asarray(inputs["vertex_indices_mask"]).astype(np.int64)
    ht = np.asarray(inputs["head_tail_indices"]).astype(np.int64)
    dis_h = np.asarray(inputs["dis_h_2_t"]).astype(np.int64)
    dis_t = np.asarray(inputs["dis_t_2_h"]).astype(np.int64)
    dis_embed = np.asarray(inputs["dis_embed"], dtype=np.float32)
    w1 = np.asarray(inputs["W1"], dtype=np.float32)
    w2 = np.asarray(inputs["W2"], dtype=np.float32)
    b2 = np.asarray(inputs["b2"], dtype=np.float32)

    # W1 row blocks: a (0:768) -> w1ac chunks 0-5, c (788:1556) -> 6-11,
    # p (1576:2344) -> w1p chunks 0-5; b (768:788) + d (1556:1576) -> w1bd.
    w1ac_rows = np.concatenate([w1[0:768], w1[788:1556]])
    w1ac_p = np.ascontiguousarray(
        w1ac_rows.astype(BF16).reshape(NKA, 128, HID).transpose(1, 0, 2))
    w1p_p = np.ascontiguousarray(
        w1[1576:2344].astype(BF16).reshape(NKP, 128, HID).transpose(1, 0, 2))
    w1bd_p = np.ascontiguousarray(
        np.concatenate([w1[768:788], w1[1556:1576]]).astype(BF16))
    w2_p = np.ascontiguousarray(
        w2.astype(BF16).reshape(NM3, 128, REL).transpose(1, 0, 2))

    in_maps = [
        _prep_core(c, sentence_repr, esi, vidx, vmask, ht, dis_h, dis_t,
                   dis_embed, w1ac_p, w1p_p, w1bd_p, w2_p, b2)
        for c in range(NCORES)
    ]

    if _NC_CACHE is None:
        _NC_CACHE = _build()

    res = bass_utils.run_bass_kernel_spmd(
        _NC_CACHE, in_maps, core_ids=list(range(NCORES)), trace=trace
    )

    out = np.empty((B, R, REL), dtype=np.float32)
    for c in range(NCORES):
        for j in range(NB):
            o = np.asarray(res.results[c][f"outd{j}"], dtype=np.float32)
            out[c * NB + j] = o[:REL].T
    return out, res


def kernel(**inputs):
    out, _ = run(**inputs)
    return out


# revision 42
# speedup vs baseline: 1.1367x; 1.0931x over previous
"""Trainium2 Bass kernel for nn_BiLSTM_M_61615600828569 (segment_reduce).

Full computation per batch:
  span_emb = masked-max-pool of token windows   (B,256,768)
  vertex_emb = masked-mean over coref spans     (B,128,768)
  head/tail  = vertex gather by relation        (B,512,768)
  feat = [head, eh, tail, et, head*tail]        (B,512,2344)
  out  = relu(feat @ W1) @ W2 + b2              (B,512,97)

Sharding: data-parallel over batch; 16 batches / 8 cores = 2 per core.

All index work happens on host, including the span-window gather itself:
for each span [s, e] the host stages 8 token rows
  [s, s+1^e, s+2^e, s+3^e, e-3|s, e-2|s, e-1|s, e]
(clamped duplicates instead of -inf masking -- every staged row is a
valid span member and their union covers [s, e] exactly for any width
0..7).  The device then reduces each span with a pure tensor_tensor max
pyramid -- no dma_gather, no gpsimd descriptor generation, no
scalar_tensor_tensor masking.

Device pipeline per batch: quad DMA chunks land -> DVE max pyramid per
128-span half -> PE: vertex pooling, V_emb^T, head/tail gather,
(V_emb @ W1) blocks, hidden accumulation (with the two distance-embed
tables stacked into a single K=40 matmul), output matmul; each batch's
output is DMA'd out as soon as it is ready.  All float math on device in
bf16 with fp32 PSUM accumulation, transposed layout (features on
partitions) so the final predict.T has the 97 classes on partitions for
a per-partition bias add.
"""
import numpy as np
import ml_dtypes
from contextlib import ExitStack

import concourse.bass as bass
import concourse.bacc as bacc
import concourse.tile as tile
from concourse import mybir
from concourse import bass_utils

BF16 = ml_dtypes.bfloat16

B, S, D = 16, 1024, 768
NS, MAXW = 256, 8
V, C = 128, 6
R = 512
REL, HID, DIS = 97, 384, 20

NCORES = 8
NB = B // NCORES          # batches per core = 2
NM = 6                    # 128-row d-chunks in D
NM3 = HID // 128          # hidden 128-row chunks = 3
NKA = 12                  # w1main chunks for a+c blocks
NKP = 6                   # w1main chunks for prod block
NKMAIN = NKA + NKP        # 18

# early const blob (bf16, lands first): [poolt | disbd | distsel | w1bd]
# rows 0..39 carry the 40-partition dis tables/weights in their own columns.
CA_POOL = 0                       # [128, NB, 2, V]        NB*2*V   = 512
CA_DISBD = CA_POOL + NB * 2 * V   # [40, 40] block-diag dis_embed^T
CA_DSEL = CA_DISBD + 40           # [40, NB, R] stacked ehsel/etsel
CA_W1BD = CA_DSEL + NB * R        # [40, HID] stacked W1b/W1d rows
CA_COLS = CA_W1BD + HID

# late const blob (bf16): [hsel | tsel | w2]
CB_HSEL = 0                       # [128, NB, R]           NB*R     = 1024
CB_TSEL = CB_HSEL + NB * R        # [128, NB, R]           NB*R     = 1024
CB_W2 = CB_TSEL + NB * R          # [128, 3, REL]          3*REL    = 291
CB_COLS = CB_W2 + NM3 * REL

F8 = ml_dtypes.float8_e4m3


def _patch_drain_and_barrier():
    """Walrus rejects >1 explicit sync wait on a Drain (TPB_CTRL), but Tile's
    tail drain waits on every used proc sem at once. Emit one single-wait
    drain per proc instead; the final drain then needs no waits."""
    import concourse.tile as tile_mod
    from concourse.vector_clock import VectorClock, ScopedClock

    if getattr(tile_mod.TileContext, "_ant_drain_patched", False):
        return

    def _patched(self, tick_clock, wait_clock):
        full = tick_clock.global_clock
        n = len(full)
        engines = [self.nc.sync, self.nc.vector, self.nc.scalar,
                   self.nc.tensor, self.nc.gpsimd]
        for i, p in enumerate([q for q in range(n) if full[q] > 0]):
            vec = [full[q] if q == p else 0 for q in range(n)]
            d = engines[i % len(engines)].drain()
            wait_clock.add_sem_waits(d.ins, ScopedClock({None: VectorClock(vec)}))
        self.nc.sync.drain()
        self.nc.all_engine_barrier()
        popped = self.nc._tile_sem_poison_stack.pop()
        assert popped is self._sem_poison
        # outermost (only) tile context at program end: skip the on-device
        # semaphore clear + second barrier; do host bookkeeping only.
        sem_nums = [s.num if hasattr(s, "num") else s
                    for s in self.sems.allocated().values()]
        self.nc._state.prepend_free_semaphores(sem_nums)

    tile_mod.TileContext._drain_and_barrier = _patched
    tile_mod.TileContext._ant_drain_patched = True


_patch_drain_and_barrier()

_NC_CACHE = None


def _build():
    """One-core program; SPMD-replicated across the 8 cores."""
    bf = mybir.dt.bfloat16
    f32 = mybir.dt.float32
    AF = mybir.ActivationFunctionType
    MAX = mybir.AluOpType.max

    nc = bacc.Bacc("TRN2", target_bir_lowering=False, debug=False, num_devices=1)

    # host-staged span windows: per (batch, span-half q, row-half) one
    # contiguous [128, 4*D] chunk; span i = q*128 + p.
    f8 = mybir.dt.float8e4
    DR = mybir.MatmulPerfMode.DoubleRow
    sq = [[[nc.dram_tensor(f"sq_{h}_{q}_{f}", (128, 4 * D), bf,
                           kind="ExternalInput")
            for f in range(2)] for q in range(2)] for h in range(NB)]
    cba = nc.dram_tensor("cba", (128, CA_COLS), bf, kind="ExternalInput")
    cbb = nc.dram_tensor("cbb", (128, CB_COLS), bf, kind="ExternalInput")
    w1ac = nc.dram_tensor("w1ac", (128, NKA, HID), bf, kind="ExternalInput")
    w1p = nc.dram_tensor("w1p", (128, NKP, HID), bf, kind="ExternalInput")
    cb32 = nc.dram_tensor("cb32", (128, NB + 1), f32, kind="ExternalInput")
    outd = [nc.dram_tensor(f"outd{b}", (128, R), f32, kind="ExternalOutput")
            for b in range(NB)]

    with tile.TileContext(nc) as tc, ExitStack() as ctx:
        consts = ctx.enter_context(tc.tile_pool(name="consts", bufs=1))
        work = ctx.enter_context(tc.tile_pool(name="work", bufs=1))
        perb = ctx.enter_context(tc.tile_pool(name="perb", bufs=2))
        psums = ctx.enter_context(tc.tile_pool(name="psums", bufs=1, space="PSUM"))

        def psum_tile(name, tag, bufs, shape=None):
            return psums.tile(shape or [128, R], mybir.dt.float32, space="PSUM",
                              tag=tag, bufs=bufs, name=name)

        # ---- input DMAs, in priority order, all issued on sync ----
        sq_t = [[[None] * 2 for _ in range(2)] for _ in range(NB)]
        for f in range(2):
            t = work.tile([128, 4 * D], bf, name=f"sq_0_0_{f}",
                          tag=f"sq_0_0_{f}")
            nc.sync.dma_start(out=t[:], in_=sq[0][0][f].ap())
            sq_t[0][0][f] = t
        cba_t = consts.tile([128, CA_COLS], bf)
        nc.sync.dma_start(out=cba_t[:], in_=cba.ap())
        cb32_t = consts.tile([128, NB + 1], f32)
        nc.sync.dma_start(out=cb32_t[:], in_=cb32.ap())
        for f in range(2):
            t = work.tile([128, 4 * D], bf, name=f"sq_0_1_{f}",
                          tag=f"sq_0_1_{f}")
            nc.sync.dma_start(out=t[:], in_=sq[0][1][f].ap())
            sq_t[0][1][f] = t
        cb_t = consts.tile([128, CB_COLS], bf)
        nc.sync.dma_start(out=cb_t[:], in_=cbb.ap())
        w1_t = consts.tile([128, NKA, HID], bf)
        nc.sync.dma_start(out=w1_t[:], in_=w1ac.ap())
        w1p_t = consts.tile([128, NKP, HID], bf)
        nc.sync.dma_start(out=w1p_t[:], in_=w1p.ap())
        for q in range(2):
            for f in range(2):
                t = work.tile([128, 4 * D], bf, name=f"sq_1_{q}_{f}",
                              tag=f"sq_1_{q}_{f}")
                nc.sync.dma_start(out=t[:], in_=sq[1][q][f].ap())
                sq_t[1][q][f] = t

        # const views
        poolt = cba_t[:, CA_POOL : CA_POOL + NB * 2 * V].rearrange(
            "p (b q v) -> p b q v", b=NB, q=2)
        disbd = cba_t[:40, CA_DISBD : CA_DISBD + 40]
        dsel = cba_t[:40, CA_DSEL : CA_DSEL + NB * R].rearrange(
            "p (b r) -> p b r", b=NB)
        w1bd_t = cba_t[:40, CA_W1BD : CA_W1BD + HID]
        hsel = cb_t[:, CB_HSEL : CB_HSEL + NB * R].rearrange(
            "p (b r) -> p b r", b=NB)
        tsel = cb_t[:, CB_TSEL : CB_TSEL + NB * R].rearrange(
            "p (b r) -> p b r", b=NB)
        w2sb = cb_t[:, CB_W2 : CB_W2 + NM3 * REL].rearrange(
            "p (k r) -> p k r", k=NM3)
        inv_t = cb32_t[:, 0:NB]
        b2_t = cb32_t[:, NB : NB + 1]

        # ---- PE warm-up: keep the tensor engine busy (and its clock
        # ramping) while the first batch's span data is still in flight.
        # Gated only on a local memset so it starts right after the
        # framework preamble, and sized to finish before the first span
        # max-pyramid completes. ----
        warm = consts.tile([40, 64], bf, name="warm")
        nc.gpsimd.memset(warm[:], 0.0)
        for _ in range(38):
            ps_w = psum_tile("ps_w", "out", 1, shape=[64, 64])
            nc.tensor.matmul(ps_w[:], lhsT=warm[:], rhs=warm[:],
                             start=True, stop=True)

        # ---- EwS = blockdiag(disT, disT) @ [W1b; W1d]  (40, HID) ----
        ps_e = psum_tile("ps_e", "out", 1, shape=[40, HID])
        nc.tensor.matmul(ps_e[:], lhsT=disbd, rhs=w1bd_t,
                         start=True, stop=True)
        ews = consts.tile([40, HID], bf, name="ews")
        nc.scalar.activation(ews[:], ps_e[:], AF.Copy)

        # ---- per-batch: span max pyramid then compute ----
        sem_b = []  # sem_b[h][p, q, :] = span_emb[q*128 + p]
        for b in range(NB):
            sh = work.tile([128, 2, D], bf, name=f"sem_{b}", tag=f"sem_{b}")
            for q in range(2):
                t1 = work.tile([128, 4 * D], bf, name=f"t1_{b}_{q}", tag="t1",
                               bufs=2)
                nc.vector.tensor_tensor(out=t1[:], in0=sq_t[b][q][0][:],
                                        in1=sq_t[b][q][1][:], op=MAX)
                nc.vector.tensor_tensor(out=t1[:, 0 : 2 * D],
                                        in0=t1[:, 0 : 2 * D],
                                        in1=t1[:, 2 * D : 4 * D], op=MAX)
                nc.vector.tensor_tensor(out=sh[:, q, :], in0=t1[:, 0:D],
                                        in1=t1[:, D : 2 * D], op=MAX)
            sem_b.append(sh)

            # vertex pooling: V_emb = poolt^T @ span_emb (then * inv)
            ps_v = psums.tile([128, D], mybir.dt.float32, space="PSUM",
                              tag="ps_v", bufs=1, name="ps_v")
            for q in range(2):
                for n0, nsz in ((0, 512), (512, 256)):
                    nc.tensor.matmul(
                        ps_v[:, n0 : n0 + nsz],
                        lhsT=poolt[:, b, q, :],
                        rhs=sem_b[b][:, q, n0 : n0 + nsz],
                        start=(q == 0), stop=(q == 1),
                    )
            v_sb = perb.tile([V, D], bf, tag="v_sb")
            nc.scalar.activation(v_sb[:], ps_v[:], AF.Copy,
                                 scale=inv_t[:, b : b + 1])

            # V_emb^T chunks (unscaled; vw applies inv)
            vt_sb = perb.tile([128, NM, V], bf, tag="vt_sb")
            for m in range(NM):
                ps_vt = psum_tile("ps_vt", "sel", 3)
                for q in range(2):
                    nc.tensor.matmul(ps_vt[:, :V],
                                     lhsT=sem_b[b][:, q, m * 128 : (m + 1) * 128],
                                     rhs=poolt[:, b, q, :],
                                     start=(q == 0), stop=(q == 1))
                nc.any.tensor_copy(vt_sb[:, m, :], ps_vt[:, :V])

            # head/tail gather + product
            head_t = perb.tile([128, NM, R], bf, tag="head_t")
            tail_t = perb.tile([128, NM, R], bf, tag="tail_t")
            prod_t = perb.tile([128, NM, R], bf, tag="prod_t")
            for m in range(NM):
                ps_h = psum_tile("ps_h", "sel", 3)
                nc.tensor.matmul(ps_h[:], lhsT=v_sb[:, m * 128 : (m + 1) * 128],
                                 rhs=hsel[:, b, :], start=True, stop=True)
                nc.any.tensor_copy(head_t[:, m, :], ps_h[:])
                ps_t2 = psum_tile("ps_t2", "sel", 3)
                nc.tensor.matmul(ps_t2[:], lhsT=v_sb[:, m * 128 : (m + 1) * 128],
                                 rhs=tsel[:, b, :], start=True, stop=True)
                nc.any.tensor_copy(tail_t[:, m, :], ps_t2[:])
                nc.vector.tensor_tensor(out=prod_t[:, m, :],
                                        in0=head_t[:, m, :],
                                        in1=tail_t[:, m, :],
                                        op=mybir.AluOpType.mult)

            # Vw_a / Vw_c = (V_emb @ W1a|W1c) * inv
            vw_a = perb.tile([V, HID], bf, tag="vw_a")
            vw_c = perb.tile([V, HID], bf, tag="vw_c")
            ps_vw_a = psum_tile("ps_vw_a", "hid", 2)
            ps_vw_c = psum_tile("ps_vw_c", "hid", 2)
            for m in range(NM):
                nc.tensor.matmul(ps_vw_a[:, :HID], lhsT=vt_sb[:, m, :],
                                 rhs=w1_t[:, m, :],
                                 start=(m == 0), stop=(m == NM - 1))
                nc.tensor.matmul(ps_vw_c[:, :HID], lhsT=vt_sb[:, m, :],
                                 rhs=w1_t[:, NM + m, :],
                                 start=(m == 0), stop=(m == NM - 1))
            nc.scalar.activation(vw_a[:], ps_vw_a[:, :HID], AF.Copy,
                                 scale=inv_t[:, b : b + 1])
            nc.scalar.activation(vw_c[:], ps_vw_c[:, :HID], AF.Copy,
                                 scale=inv_t[:, b : b + 1])

            # hidden = relu( vw_a[h] + vw_c[t] + EwS-gather + W1p^T prod )
            hid_t = perb.tile([128, NM3, R], bf, tag="hid_t")
            for m3 in range(NM3):
                msl = slice(m3 * 128, (m3 + 1) * 128)
                ps_hid = psum_tile("ps_hid", "hid", 2)
                nc.tensor.matmul(ps_hid[:], lhsT=ews[:, msl], rhs=dsel[:, b, :],
                                 start=True, stop=False)
                nc.tensor.matmul(ps_hid[:], lhsT=vw_a[:, msl], rhs=hsel[:, b, :],
                                 start=False, stop=False)
                nc.tensor.matmul(ps_hid[:], lhsT=vw_c[:, msl], rhs=tsel[:, b, :],
                                 start=False, stop=False)
                for m in range(NM):
                    nc.tensor.matmul(ps_hid[:], lhsT=w1p_t[:, m, msl],
                                     rhs=prod_t[:, m, :],
                                     start=False, stop=(m == NM - 1))
                nc.scalar.activation(hid_t[:, m3, :], ps_hid[:], AF.Relu)

            # out = W2^T @ hidden + b2, classes on partitions
            out_sb = perb.tile([128, R], f32, tag="out_sb")
            ps_o = psum_tile("ps_o", "out", 1)
            for kc in range(NM3):
                nc.tensor.matmul(ps_o[:REL, :], lhsT=w2sb[:, kc, :],
                                 rhs=hid_t[:, kc, :],
                                 start=(kc == 0), stop=(kc == NM3 - 1))
            nc.scalar.activation(out_sb[:REL, :], ps_o[:REL, :], AF.Identity,
                                 bias=b2_t[:REL, 0:1])
            nc.sync.dma_start(out=outd[b].ap(), in_=out_sb[:])

    nc.compile()
    return nc


def _prep_core(c, sentence_repr, esi, vidx, vmask, ht, dis_h, dis_t,
               dis_embed, w1ac_p, w1p_p, w1bd_p, w2_p, b2):
    """Build the per-core input map for batches [c*NB, c*NB+NB)."""
    bs = range(c * NB, c * NB + NB)
    inputs = {"w1ac": w1ac_p, "w1p": w1p_p}

    poolt = np.zeros((128, NB, 2, V), dtype=BF16)
    hsel = np.zeros((V, NB, R), dtype=BF16)
    tsel = np.zeros((V, NB, R), dtype=BF16)
    dsel = np.zeros((40, NB, R), dtype=BF16)
    invcnt = np.zeros((V, NB), dtype=np.float32)
    rr = np.arange(R)

    for j, b in enumerate(bs):
        # staged span windows: 8 valid (clamped-duplicate) rows per span
        s = esi[b, :, 0]
        e = esi[b, :, 1]
        rows = np.stack([s,
                         np.minimum(s + 1, e),
                         np.minimum(s + 2, e),
                         np.minimum(s + 3, e),
                         np.maximum(e - 3, s),
                         np.maximum(e - 2, s),
                         np.maximum(e - 1, s),
                         e], axis=1)                     # (NS, 8)
        gath = sentence_repr[b][rows]                    # (NS, 8, D) f32
        gath = gath.reshape(2, 128, 2, 4 * D).astype(BF16)   # q, p, f, :
        for q in range(2):
            for f in range(2):
                inputs[f"sq_{j}_{q}_{f}"] = np.ascontiguousarray(gath[q, :, f])

        pt = np.zeros((NS, V), dtype=np.float32)
        np.add.at(pt, (vidx[b].ravel(), np.repeat(np.arange(V), C)),
                  vmask[b].ravel().astype(np.float32))
        poolt[:, j] = pt.reshape(2, 128, V).transpose(1, 0, 2).astype(BF16)
        invcnt[:, j] = 1.0 / np.maximum(vmask[b].sum(axis=1).astype(np.float32), 1.0)
        hsel[ht[b, :, 0], j, rr] = BF16(1.0)
        tsel[ht[b, :, 1], j, rr] = BF16(1.0)
        dsel[dis_h[b], j, rr] = BF16(1.0)
        dsel[20 + dis_t[b], j, rr] = BF16(1.0)

    cba = np.zeros((128, CA_COLS), dtype=BF16)
    cba[:, CA_POOL : CA_POOL + NB * 2 * V] = poolt.reshape(128, -1)
    cba[:20, CA_DISBD : CA_DISBD + 20] = dis_embed.T.astype(BF16)
    cba[20:40, CA_DISBD + 20 : CA_DISBD + 40] = dis_embed.T.astype(BF16)
    cba[:40, CA_DSEL : CA_DSEL + NB * R] = dsel.reshape(40, -1)
    cba[:40, CA_W1BD : CA_W1BD + HID] = w1bd_p
    inputs["cba"] = cba

    cbb = np.zeros((128, CB_COLS), dtype=BF16)
    cbb[:V, CB_HSEL : CB_HSEL + NB * R] = hsel.reshape(V, -1)
    cbb[:V, CB_TSEL : CB_TSEL + NB * R] = tsel.reshape(V, -1)
    cbb[:, CB_W2 : CB_W2 + NM3 * REL] = w2_p.reshape(128, -1)
    inputs["cbb"] = cbb

    cb32 = np.zeros((128, NB + 1), dtype=np.float32)
    cb32[:V, 0:NB] = invcnt
    cb32[:REL, NB] = b2
    inputs["cb32"] = cb32
    return inputs


def run(trace=False, **inputs):
    global _NC_CACHE
    sentence_repr = np.asarray(inputs["sentence_repr"], dtype=np.float32)
    esi = np.asarray(inputs["entity_span_indices"]).astype(np.int64)
    vidx = np.asarray(inputs["vertex_indices"]).astype(np.int64)
    vmask = np.asarray(inputs["vertex_indices_mask"]).astype(np.int64)
    ht = np.asarray(inputs["head_tail_indices"]).astype(np.int64)
    dis_h = np.asarray(inputs["dis_h_2_t"]).astype(np.int64)
    dis_t = np.asarray(inputs["dis_t_2_h"]).astype(np.int64)
    dis_embed = np.asarray(inputs["dis_embed"], dtype=np.float32)
    w1 = np.asarray(inputs["W1"], dtype=np.float32)
    w2 = np.asarray(inputs["W2"], dtype=np.float32)
    b2 = np.asarray(inputs["b2"], dtype=np.float32)

    # W1 row blocks: a (0:768) -> w1ac chunks 0-5, c (788:1556) -> 6-11,
    # p (1576:2344) -> w1p chunks 0-5; b (768:788) + d (1556:1576) -> w1bd.
    w1ac_rows = np.concatenate([w1[0:768], w1[788:1556]])
    w1ac_p = np.ascontiguousarray(
        w1ac_rows.astype(BF16).reshape(NKA, 128, HID).transpose(1, 0, 2))
    w1p_p = np.ascontiguousarray(
        w1[1576:2344].astype(BF16).reshape(NKP, 128, HID).transpose(1, 0, 2))
    w1bd_p = np.ascontiguousarray(
        np.concatenate([w1[768:788], w1[1556:1576]]).astype(BF16))
    w2_p = np.ascontiguousarray(
        w2.astype(BF16).reshape(NM3, 128, REL).transpose(1, 0, 2))

    in_maps = [
        _prep_core(c, sentence_repr, esi, vidx, vmask, ht, dis_h, dis_t,
                   dis_embed, w1ac_p, w1p_p, w1bd_p, w2_p, b2)
        for c in range(NCORES)
    ]

    if _NC_CACHE is None:
        _NC_CACHE = _build()

    res = bass_utils.run_bass_kernel_spmd(
        _NC_CACHE, in_maps, core_ids=list(range(NCORES)), trace=trace
    )

    out = np.empty((B, R, REL), dtype=np.float32)
    for c in range(NCORES):
        for j in range(NB):
            o = np.asarray(res.results[c][f"outd{j}"], dtype=np.float32)
            out[c * NB + j] = o[:REL].T
    return out, res


def kernel(**inputs):
    out, _ = run(**inputs)
    return out
